# revision 1
# baseline (speedup 1.0000x reference)
"""Trainium2 Bass kernel for the GRU autoencoder.

Distribution strategy (8 NeuronCores):
  Encode : chain-parallel x batch-parallel. Core j handles GRU chain j//2
           (xf, xb, ef, eb) on batch half j%2 (128 rows), running a uniform
           100-step loop. The 50-step x-chains get 50 leading exact identity
           steps (update-gate pre-activation forced to +BIG => z=1 => h'=h).
           Input-side gates, biases and the pad flag ride the same matmul via
           augmented stationary rows (features + ones-row + flag-row).
  Reshard: AllToAll moves 16-row slices so every core assembles the hidden
           states for its own 32-row decode shard at *fixed* (SPMD-uniform)
           indices: core j decodes global rows [16j:16j+16] u [128+16j:+16].
  Middle : per-shard MLP (em1/em2/eo) + decoder const precompute (bf16).
  Decode : 60 autoregressive steps per core on its 32-row shard (fp32r).

All GRU-path matmuls use fp32r (full PE rate at N>=256, ~1e-4 rounding); the
middle MLP uses bf16 weights. PSUM accumulates in fp32 throughout.

PSUM budget (8 banks): gA 2 | gB 2x1 | gC 2x1 | tr 2x1.
"""

import sys

sys.path.insert(0, "/opt/trn_rl_repo")

import numpy as np

import concourse.bass as bass
import concourse.mybir as mybir
import concourse.tile as tile
from concourse import bacc
from concourse.masks import make_identity

dt = mybir.dt
AF = mybir.ActivationFunctionType
OP = mybir.AluOpType

B, TX, TY, NX, NY, H, HOR = 256, 50, 100, 64, 64, 512, 60
M1, M2 = 1024, 512
G = 3 * H
NCORE = 8
BE = 128   # encoder batch rows per core
BD = 32    # decoder batch rows per core
BIG = 30000.0

F32, F32R, BF16 = dt.float32, dt.float32r, dt.bfloat16


def build_nc(et=100, x_real=50, hor=60):
    nc = bacc.Bacc("TRN2", target_bir_lowering=False, debug=False,
                   num_devices=NCORE)

    # ---- DRAM parameters (identical names on every core; content differs) --
    d_xin = nc.dram_tensor("xin", [66, et * BE], F32, kind="ExternalInput")
    d_wih = nc.dram_tensor("wih_aug", [66, G], F32, kind="ExternalInput")
    d_whh = nc.dram_tensor("whh_t", [H, G], F32, kind="ExternalInput")
    d_bhhn = nc.dram_tensor("bhhn_row", [1, H], F32, kind="ExternalInput")

    d_em1 = nc.dram_tensor("em_w1t", [2 * H, M1], F32, kind="ExternalInput")
    d_em1b = nc.dram_tensor("em_b1row", [1, M1], F32, kind="ExternalInput")
    d_em2 = nc.dram_tensor("em_w2t", [M1, M2], F32, kind="ExternalInput")
    d_em2b = nc.dram_tensor("em_b2row", [1, M2], F32, kind="ExternalInput")
    d_eow = nc.dram_tensor("eo_wt", [M2, H], F32, kind="ExternalInput")
    d_eob = nc.dram_tensor("eo_brow", [1, H], F32, kind="ExternalInput")
    d_dcw = nc.dram_tensor("dc_wt", [2 * H, G], F32, kind="ExternalInput")
    d_dcb = nc.dram_tensor("dc_brow", [1, G], F32, kind="ExternalInput")

    d_dwy = nc.dram_tensor("dwy_t", [NY, G], F32, kind="ExternalInput")
    d_dwhh = nc.dram_tensor("dwhh_t", [H, G], F32, kind="ExternalInput")
    d_dbhhn = nc.dram_tensor("dbhhn_row", [1, H], F32, kind="ExternalInput")
    d_dm1 = nc.dram_tensor("dm_w1t", [H, M1], F32, kind="ExternalInput")
    d_dm1b = nc.dram_tensor("dm_b1row", [1, M1], F32, kind="ExternalInput")
    d_dm2 = nc.dram_tensor("dm_w2t", [M1, M2], F32, kind="ExternalInput")
    d_dm2b = nc.dram_tensor("dm_b2row", [1, M2], F32, kind="ExternalInput")
    d_dow = nc.dram_tensor("do_wt", [M2, NY], F32, kind="ExternalInput")
    d_dob = nc.dram_tensor("do_brow", [1, NY], F32, kind="ExternalInput")
    d_xlast = nc.dram_tensor("xlast_t", [NX, BD], F32, kind="ExternalInput")

    d_out = nc.dram_tensor("out", [BD, hor * NY], F32, kind="ExternalOutput")

    cc_in = nc.dram_tensor("cc_in", [BE, H], F32)
    cc_out = nc.dram_tensor("cc_out", [NCORE, 16, H], F32)

    with tile.TileContext(nc) as tc:
        with tc.tile_pool(name="wenc", bufs=1) as wenc, \
             tc.tile_pool(name="wstream", bufs=2) as ws, \
             tc.tile_pool(name="wdec", bufs=1) as wdec, \
             tc.tile_pool(name="state", bufs=2) as st, \
             tc.tile_pool(name="tmp", bufs=2) as tp, \
             tc.tile_pool(name="tmp1", bufs=1) as tq, \
             tc.tile_pool(name="mid", bufs=1) as md, \
             tc.tile_pool(name="ldtmp", bufs=2) as ld, \
             tc.tile_pool(name="persist", bufs=1) as pe, \
             tc.tile_pool(name="psA", bufs=2, space="PSUM") as psA, \
             tc.tile_pool(name="psB", bufs=1, space="PSUM") as psB, \
             tc.tile_pool(name="psC", bufs=2, space="PSUM") as psC, \
             tc.tile_pool(name="psTR", bufs=1, space="PSUM") as psTR:

            # ---------- constants ----------
            idf = pe.tile([128, 128], F32, tag="idf")
            make_identity(nc, idf[:])
            id32 = idf[0:32, 0:32]
            ones_f = pe.tile([1, 128], F32, tag="ones_f")
            nc.gpsimd.memset(ones_f[:], 1.0)
            ones_r = pe.tile([1, 128], F32R, tag="ones_r")
            nc.vector.tensor_copy(ones_r[:], ones_f[:])
            zero_f = pe.tile([128, 128], F32, tag="zero_f")
            nc.gpsimd.memset(zero_f[:], 0.0)
            id_r = pe.tile([32, 32], F32R, tag="id_r")
            nc.vector.tensor_copy(id_r[:], id32)
            ones_b = pe.tile([1, 128], BF16, tag="ones_b")
            nc.gpsimd.tensor_copy(ones_b[:], ones_f[:])

            def load_r(pool, dram_ap, rows, cols, tag, rdt=F32R, eng=None):
                r = pool.tile([rows, cols], rdt, tag=tag)
                for lo in range(0, cols, 768):
                    hi = min(cols, lo + 768)
                    t = ld.tile([rows, hi - lo], F32, tag="ldtmp")
                    nc.sync.dma_start(t[:], dram_ap[:, lo:hi])
                    (eng or nc.gpsimd).tensor_copy(r[:, lo:hi], t[:])
                return r

            # ---------- encoder weights (needed immediately; DVE rounds) ----
            wih_r = load_r(wenc, d_wih[:], 66, G, "wih", eng=nc.vector)
            whh_r = [load_r(wenc, d_whh[128 * c:128 * (c + 1), :], 128, G,
                            f"whh{c}", eng=nc.vector) for c in range(4)]

            # ---------- encoder state ----------
            hT = pe.tile([128, H], F32R, tag="hT0")
            for c in range(4):
                nc.vector.tensor_copy(hT[:, 128 * c:128 * (c + 1)], zero_f[:])
            h_bh = pe.tile([BE, H], F32, tag="h0")
            nc.gpsimd.memset(h_bh[:], 0.0)

            # ---------- middle/decoder weights (gpsimd rounds in background)
            # column-packed bf16 biases: dcb|em1b|em2b|eob
            bias_b = pe.tile([1, 3584], BF16, tag="bias_b")
            for dtn, base, w in ((d_dcb, 0, G), (d_em1b, G, M1),
                                 (d_em2b, G + M1, M2), (d_eob, 3072, M2)):
                for lo in range(0, w, 768):
                    hi = min(w, lo + 768)
                    t = ld.tile([1, hi - lo], F32, tag="ldtmp")
                    nc.sync.dma_start(t[:], dtn[:, lo:hi])
                    nc.gpsimd.tensor_copy(bias_b[0:1, base + lo:base + hi],
                                          t[:])

            dwyc = wdec.tile([96, G], F32R, tag="dwyc")
            for lo in range(0, G, 768):
                hi = lo + 768
                t = ld.tile([NY, 768], F32, tag="ldtmp")
                nc.sync.dma_start(t[:], d_dwy[:, lo:hi])
                nc.gpsimd.tensor_copy(dwyc[0:NY, lo:hi], t[:])
            # identity block staged at partitions 64:96 (for ypT_aug rows)
            id_hi_f = pe.tile([96, 32], F32, tag="id_hi_f")
            nc.sync.dma_start(id_hi_f[64:96, :], idf[0:32, 0:32])
            dwhh_r = [load_r(wdec, d_dwhh[128 * c:128 * (c + 1), :], 128, G,
                             f"dwhh{c}") for c in range(4)]
            dm1_r = [load_r(wdec, d_dm1[128 * c:128 * (c + 1), :], 128, M1,
                            f"dm1_{c}") for c in range(4)]
            dm2_r = [load_r(wdec, d_dm2[128 * c:128 * (c + 1), :], 128, M2,
                            f"dm2_{c}") for c in range(8)]
            dow_r = [load_r(wdec, d_dow[128 * c:128 * (c + 1), :], 128, NY,
                            f"dow_{c}", BF16) for c in range(4)]
            # column-packed f32r biases: bhhn|dbhhn|dm1b|dm2b|dob
            bias_r = pe.tile([1, 2624], F32R, tag="bias_r")
            for dtn, base, w in ((d_bhhn, 0, H), (d_dbhhn, H, H),
                                 (d_dm1b, 1024, M1), (d_dm2b, 2048, M2),
                                 (d_dob, 2560, NY)):
                for lo in range(0, w, 768):
                    hi = min(w, lo + 768)
                    t = ld.tile([1, hi - lo], F32, tag="ldtmp")
                    nc.sync.dma_start(t[:], dtn[:, lo:hi])
                    nc.gpsimd.tensor_copy(bias_r[0:1, base + lo:base + hi],
                                          t[:])
            xlast_r = load_r(wdec, d_xlast[:], NX, BD, "xlastr")

            # ---------- encode loop ----------
            for t in range(et):
                xs_f = tp.tile([66, 128], F32, tag="xs_f")
                nc.sync.dma_start(xs_f[:], d_xin[:, t * BE:(t + 1) * BE])
                xs = tp.tile([66, 128], F32R, tag="xs_r")
                nc.vector.tensor_copy(xs[:], xs_f[:])

                g1a = psA.tile([BE, 512], F32, tag="gA1")
                g1b = psA.tile([BE, 512], F32, tag="gA2")
                g2 = psB.tile([BE, 512], F32, tag="gB")
                g3 = psC.tile([BE, 512], F32, tag="gC")
                # bank-grouped: finish each PSUM bank before switching
                nc.tensor.matmul(g1a[:], xs[:], wih_r[:, 0:512],
                                 start=True, stop=False)
                for c in range(4):
                    nc.tensor.matmul(g1a[:], hT[:, 128 * c:128 * (c + 1)],
                                     whh_r[c][:, 0:512],
                                     start=False, stop=(c == 3))
                nc.tensor.matmul(g1b[:], xs[:], wih_r[:, 512:1024],
                                 start=True, stop=False)
                for c in range(4):
                    nc.tensor.matmul(g1b[:], hT[:, 128 * c:128 * (c + 1)],
                                     whh_r[c][:, 512:1024],
                                     start=False, stop=(c == 3))
                for c in range(4):
                    nc.tensor.matmul(g2[:], hT[:, 128 * c:128 * (c + 1)],
                                     whh_r[c][:, 1024:1536],
                                     start=(c == 0), stop=False)
                nc.tensor.matmul(g2[:], ones_r[0:1, 0:BE],
                                 bias_r[0:1, 0:512], start=False, stop=True)
                nc.tensor.matmul(g3[:], xs[:], wih_r[:, 1024:1536],
                                 start=True, stop=True)

                r_t = tp.tile([BE, 512], F32, tag="r")
                z_t = tp.tile([BE, 512], F32, tag="z")
                omz = tp.tile([BE, 512], F32, tag="omz")
                nc.scalar.activation(r_t[:], g1a[:], AF.Sigmoid)
                nc.scalar.activation(z_t[:], g1b[:], AF.Sigmoid)
                nc.scalar.activation(omz[:], g1b[:], AF.Sigmoid,
                                     scale=-1.0)
                rhn = tp.tile([BE, 512], F32, tag="rhn")
                nc.vector.tensor_mul(rhn[:], r_t[:], g2[:])
                npre = tp.tile([BE, 512], F32, tag="npre")
                nc.vector.tensor_add(npre[:], rhn[:], g3[:])
                n_t = tp.tile([BE, 512], F32, tag="n")
                nc.scalar.activation(n_t[:], npre[:], AF.Tanh)
                a_t = tp.tile([BE, 512], F32, tag="a")
                nc.vector.tensor_mul(a_t[:], omz[:], n_t[:])
                b_t = tp.tile([BE, 512], F32, tag="b")
                nc.vector.tensor_mul(b_t[:], z_t[:], h_bh[:])
                h_new = st.tile([BE, H], F32, tag="h")
                nc.vector.tensor_add(h_new[:], a_t[:], b_t[:])

                ptr = psTR.tile([128, 512], F32, tag="tr")
                for c in range(4):
                    nc.tensor.transpose(ptr[:, 128 * c:128 * (c + 1)],
                                        h_new[:, 128 * c:128 * (c + 1)],
                                        idf[:])
                hT_new = st.tile([128, H], F32R, tag="hT")
                nc.scalar.copy(hT_new[:], ptr[:])
                hT, h_bh = hT_new, h_new

            # ---------- reshard: AllToAll of 16-row slices ----------
            nc.sync.dma_start(cc_in[:], h_bh[:])
            nc.gpsimd.collective_compute(
                "AllToAll", OP.bypass,
                replica_groups=[list(range(NCORE))],
                ins=[cc_in[:]], outs=[cc_out[:]])

            pxa = md.tile([BD, H], F32, tag="pA")
            pxb = md.tile([BD, H], F32, tag="pB")
            pya = md.tile([BD, H], F32, tag="pA")
            pyb = md.tile([BD, H], F32, tag="pB")
            nc.sync.dma_start(pxa[0:16, :], cc_out[0][:])
            nc.sync.dma_start(pxa[16:32, :], cc_out[1][:])
            nc.sync.dma_start(pxb[0:16, :], cc_out[2][:])
            nc.sync.dma_start(pxb[16:32, :], cc_out[3][:])
            nc.sync.dma_start(pya[0:16, :], cc_out[4][:])
            nc.sync.dma_start(pya[16:32, :], cc_out[5][:])
            nc.sync.dma_start(pyb[0:16, :], cc_out[6][:])
            nc.sync.dma_start(pyb[16:32, :], cc_out[7][:])
            hx = md.tile([BD, H], F32, tag="hx")
            hy = md.tile([BD, H], F32, tag="hy")
            nc.vector.tensor_add(hx[:], pxa[:], pxb[:])
            nc.vector.tensor_add(hy[:], pya[:], pyb[:])

            def trsp_b(src, cols, tag):
                """src [BD, cols] f32 -> bf16 [128, (cols//128)*BD]."""
                nch = cols // 128
                p = psTR.tile([128, nch * BD], F32, tag="tr")
                for c in range(nch):
                    nc.tensor.transpose(p[:, BD * c:BD * (c + 1)],
                                        src[:, 128 * c:128 * (c + 1)], id32)
                o = pe.tile([128, nch * BD], BF16, tag=tag)
                nc.scalar.copy(o[:], p[:])
                return o

            hxT = trsp_b(hx, H, "hxT")
            hyT = trsp_b(hy, H, "hyT")

            m1a = psA.tile([BD, 512], F32, tag="gA1")
            m1b = psA.tile([BD, 512], F32, tag="gA2")
            for c in range(8):
                wt = load_r(ws, d_em1[128 * c:128 * (c + 1), :], 128, M1,
                            f"wstr{c % 2}", BF16)
                s = (hxT if c < 4 else hyT)[:, BD * (c % 4):BD * (c % 4 + 1)]
                nc.tensor.matmul(m1a[:], s, wt[:, 0:512],
                                 start=(c == 0), stop=False)
                nc.tensor.matmul(m1b[:], s, wt[:, 512:1024],
                                 start=(c == 0), stop=False)
            nc.tensor.matmul(m1a[:], ones_b[0:1, 0:BD],
                             bias_b[0:1, 1536:2048], start=False, stop=True)
            nc.tensor.matmul(m1b[:], ones_b[0:1, 0:BD],
                             bias_b[0:1, 2048:2560], start=False, stop=True)
            hm1 = tq.tile([BD, M1], F32, tag="hm1")
            nc.scalar.activation(hm1[:, 0:512], m1a[:], AF.Relu)
            nc.scalar.activation(hm1[:, 512:1024], m1b[:], AF.Relu)
            hm1T = trsp_b(hm1, M1, "hm1T_m")

            m2 = psB.tile([BD, M2], F32, tag="gB")
            for c in range(8):
                wt = load_r(ws, d_em2[128 * c:128 * (c + 1), :], 128, M2,
                            f"wstr{c % 2}", BF16)
                nc.tensor.matmul(m2[:], hm1T[:, BD * c:BD * (c + 1)],
                                 wt[:], start=(c == 0), stop=False)
            nc.tensor.matmul(m2[:], ones_b[0:1, 0:BD], bias_b[0:1, 2560:3072],
                             start=False, stop=True)
            hm2 = tq.tile([BD, M2], F32, tag="hm2")
            nc.scalar.activation(hm2[:], m2[:], AF.Relu)
            hm2T = trsp_b(hm2, M2, "hm2T_m")

            zp = psC.tile([BD, H], F32, tag="gC")
            for c in range(4):
                wt = load_r(ws, d_eow[128 * c:128 * (c + 1), :], 128, H,
                            f"wstr{c % 2}", BF16)
                nc.tensor.matmul(zp[:], hm2T[:, BD * c:BD * (c + 1)],
                                 wt[:], start=(c == 0), stop=False)
            nc.tensor.matmul(zp[:], ones_b[0:1, 0:BD], bias_b[0:1, 3072:3584],
                             start=False, stop=True)
            z_sb = md.tile([BD, H], F32, tag="z_sb")
            nc.scalar.copy(z_sb[:], zp[:])
            zT = trsp_b(z_sb, H, "zT")

            # const = cat(h_x, z) @ d_Wih[:, :2H].T + d_bih + d_bhh(r,z)
            cpa = psA.tile([96, 512], F32, tag="gA1")
            cpa = cpa[64:96, :]
            cpb = psA.tile([96, 512], F32, tag="gA2")
            cpb = cpb[64:96, :]
            cpn = psB.tile([96, 512], F32, tag="gB")
            cpn = cpn[64:96, :]
            for c in range(8):
                wt = load_r(ws, d_dcw[128 * c:128 * (c + 1), :], 128, G,
                            f"wstr{c % 2}", BF16)
                s = (hxT if c < 4 else zT)[:, BD * (c % 4):BD * (c % 4 + 1)]
                nc.tensor.matmul(cpa[:], s, wt[:, 0:512],
                                 start=(c == 0), stop=False)
                nc.tensor.matmul(cpb[:], s, wt[:, 512:1024],
                                 start=(c == 0), stop=False)
                nc.tensor.matmul(cpn[:], s, wt[:, 1024:1536],
                                 start=(c == 0), stop=False)
            nc.tensor.matmul(cpa[:], ones_b[0:1, 0:BD],
                             bias_b[0:1, 0:512], start=False, stop=True)
            nc.tensor.matmul(cpb[:], ones_b[0:1, 0:BD],
                             bias_b[0:1, 512:1024], start=False, stop=True)
            nc.tensor.matmul(cpn[:], ones_b[0:1, 0:BD],
                             bias_b[0:1, 1024:1536], start=False, stop=True)
            nc.vector.tensor_copy(dwyc[64:96, 0:512], cpa[:])
            nc.vector.tensor_copy(dwyc[64:96, 512:1024], cpb[:])
            nc.vector.tensor_copy(dwyc[64:96, 1024:1536], cpn[:])

            # decoder init
            hdT = st.tile([128, 4 * BD], F32R, tag="hdT")
            nc.vector.tensor_copy(hdT[:], zero_f[:])
            hd = st.tile([BD, H], F32, tag="hd")
            nc.gpsimd.memset(hd[:], 0.0)
            ypT = st.tile([96, BD], F32R, tag="ypT")
            nc.vector.tensor_copy(ypT[0:NX, :], xlast_r[:])
            nc.vector.tensor_copy(ypT[64:96, :], id_hi_f[64:96, :])

            # ---------- decode loop ----------
            for t in range(hor):
                g1a = psA.tile([BD, 512], F32, tag="gA1")
                g1b = psA.tile([BD, 512], F32, tag="gA2")
                g2 = psB.tile([BD, 512], F32, tag="gB")
                g3 = psC.tile([BD, 512], F32, tag="gC")
                # h-side first: depends only on hdT (ready since last GRU
                # phase), so these stream during the previous step's MLP.
                # The yp/const matmuls close each group once ypT lands.
                for c in range(4):
                    nc.tensor.matmul(g1a[:], hdT[:, BD * c:BD * (c + 1)],
                                     dwhh_r[c][:, 0:512],
                                     start=(c == 0), stop=False)
                for c in range(4):
                    nc.tensor.matmul(g2[:], hdT[:, BD * c:BD * (c + 1)],
                                     dwhh_r[c][:, 1024:1536],
                                     start=(c == 0), stop=False)
                nc.tensor.matmul(g2[:], ones_r[0:1, 0:BD],
                                 bias_r[0:1, 512:1024], start=False, stop=True)
                for c in range(4):
                    nc.tensor.matmul(g1b[:], hdT[:, BD * c:BD * (c + 1)],
                                     dwhh_r[c][:, 512:1024],
                                     start=(c == 0), stop=False)
                nc.tensor.matmul(g1a[:], ypT[:], dwyc[:, 0:512],
                                 start=False, stop=True)
                nc.tensor.matmul(g3[:], ypT[:], dwyc[:, 1024:1536],
                                 start=True, stop=True)
                nc.tensor.matmul(g1b[:], ypT[:], dwyc[:, 512:1024],
                                 start=False, stop=True)

                r_t = tp.tile([BD, 512], F32, tag="r")
                z_t = tp.tile([BD, 512], F32, tag="z")
                omz = tp.tile([BD, 512], F32, tag="omz")
                nc.scalar.activation(r_t[:], g1a[:], AF.Sigmoid)
                nc.scalar.activation(z_t[:], g1b[:], AF.Sigmoid)
                nc.scalar.activation(omz[:], g1b[:], AF.Sigmoid,
                                     scale=-1.0)
                rhn = tp.tile([BD, 512], F32, tag="rhn")
                nc.vector.tensor_mul(rhn[:], r_t[:], g2[:])
                npre = tp.tile([BD, 512], F32, tag="npre")
                nc.vector.tensor_add(npre[:], rhn[:], g3[:])
                n_t = tp.tile([BD, 512], F32, tag="n")
                nc.scalar.activation(n_t[:], npre[:], AF.Tanh)
                a_t = tp.tile([BD, 512], F32, tag="a")
                nc.vector.tensor_mul(a_t[:], omz[:], n_t[:])
                b_t = tp.tile([BD, 512], F32, tag="b")
                nc.gpsimd.tensor_mul(b_t[:], z_t[:], hd[:])
                hd_new = st.tile([BD, H], F32, tag="hd")
                nc.vector.tensor_add(hd_new[:], a_t[:], b_t[:])

                ptr = psTR.tile([128, 4 * BD], F32, tag="tr")
                for c in range(4):
                    nc.tensor.transpose(ptr[:, BD * c:BD * (c + 1)],
                                        hd_new[:, 128 * c:128 * (c + 1)],
                                        id32)
                hdT_new = st.tile([128, 4 * BD], F32R, tag="hdT")
                nc.scalar.copy(hdT_new[:], ptr[:])
                hdT, hd = hdT_new, hd_new

                m1a = psA.tile([BD, 512], F32, tag="gA1")
                m1b = psA.tile([BD, 512], F32, tag="gA2")
                for c in range(4):
                    nc.tensor.matmul(m1a[:], hdT[:, BD * c:BD * (c + 1)],
                                     dm1_r[c][:, 0:512],
                                     start=(c == 0), stop=False)
                nc.tensor.matmul(m1a[:], ones_r[0:1, 0:BD],
                                 bias_r[0:1, 1024:1536], start=False, stop=True)
                hm1 = tq.tile([BD, M1], F32, tag="hm1")
                nc.scalar.activation(hm1[:, 0:512], m1a[:], AF.Relu)
                for c in range(4):
                    nc.tensor.matmul(m1b[:],
                                     hdT[:, BD * c:BD * (c + 1)],
                                     dm1_r[c][:, 512:1024],
                                     start=(c == 0), stop=False)
                nc.tensor.matmul(m1b[:], ones_r[0:1, 0:BD],
                                 bias_r[0:1, 1536:2048], start=False, stop=True)
                nc.scalar.activation(hm1[:, 512:1024], m1b[:], AF.Relu)
                hm1Ta = tq.tile([128, 4 * BD], F32R, tag="hm1Ta")
                hm1Tb = tq.tile([128, 4 * BD], F32R, tag="hm1Tb")
                p1 = psTR.tile([128, 4 * BD], F32, tag="tr")
                for c in range(4):
                    nc.tensor.transpose(p1[:, BD * c:BD * (c + 1)],
                                        hm1[:, 128 * c:128 * (c + 1)], id32)
                nc.vector.tensor_copy(hm1Ta[:], p1[:])
                p1b = psTR.tile([128, 4 * BD], F32, tag="tr")
                for c in range(4):
                    nc.tensor.transpose(p1b[:, BD * c:BD * (c + 1)],
                                        hm1[:, 512 + 128 * c:640 + 128 * c],
                                        id32)
                nc.vector.tensor_copy(hm1Tb[:], p1b[:])

                m2 = psB.tile([BD, M2], F32, tag="gB")
                for c in range(8):
                    s = (hm1Ta if c < 4 else hm1Tb)[:, BD * (c % 4):
                                                    BD * (c % 4 + 1)]
                    nc.tensor.matmul(m2[:], s, dm2_r[c][:],
                                     start=(c == 0), stop=False)
                nc.tensor.matmul(m2[:], ones_r[0:1, 0:BD],
                                 bias_r[0:1, 2048:2560], start=False, stop=True)
                hm2 = tq.tile([BD, M2], F32, tag="hm2")
                nc.scalar.activation(hm2[:], m2[:], AF.Relu)
                p2 = psTR.tile([128, 4 * BD], F32, tag="tr")
                for c in range(4):
                    nc.tensor.transpose(p2[:, BD * c:BD * (c + 1)],
                                        hm2[:, 128 * c:128 * (c + 1)], id32)
                hm2T = tq.tile([128, 4 * BD], BF16, tag="hm2T")
                nc.vector.tensor_copy(hm2T[:], p2[:])

                yp_ps = psC.tile([BD, NY], F32, tag="gC")
                for c in range(4):
                    nc.tensor.matmul(yp_ps[:], hm2T[:, BD * c:BD * (c + 1)],
                                     dow_r[c][:], start=(c == 0), stop=False)
                nc.tensor.matmul(yp_ps[:], ones_r[0:1, 0:BD],
                                 bias_r[0:1, 2560:2624],
                                 start=False, stop=True)
                y_sb = tp.tile([BD, NY], F32, tag="y_sb")
                nc.scalar.copy(y_sb[:], yp_ps[:])
                nc.sync.dma_start(d_out[:, NY * t:NY * (t + 1)], y_sb[:])
                if t + 1 < hor:
                    p3 = psTR.tile([NX, BD], F32, tag="tr")
                    nc.tensor.transpose(p3[:], y_sb[:], id32)
                    ypT_new = st.tile([96, BD], F32R, tag="ypT")
                    nc.scalar.copy(ypT_new[0:NX, :], p3[:])
                    nc.vector.tensor_copy(ypT_new[64:96, :],
                                          id_hi_f[64:96, :])
                    ypT = ypT_new

    nc.compile()
    return nc


# ---------------------------------------------------------------------------
# Host-side sharding
# ---------------------------------------------------------------------------

def shard_inputs(inp, et=100, x_real=50, hor=60):
    f32 = np.float32
    x, y = np.asarray(inp["x"], f32), np.asarray(inp["y"], f32)
    tx = x.shape[1]
    chains = [("xf", False, x), ("xb", True, x),
              ("ef", False, y), ("eb", True, y)]
    in_maps = []
    shared = {}

    def wih_aug(pre):
        wih = np.asarray(inp[pre + "_Wih"], f32)
        bih = np.asarray(inp[pre + "_bih"], f32)
        bhh = np.asarray(inp[pre + "_bhh"], f32)
        aug = np.zeros((66, G), f32)
        aug[0:64, :] = wih.T
        bias = bih.copy()
        bias[0:2 * H] += bhh[0:2 * H]
        aug[64, :] = bias
        aug[65, H:2 * H] = BIG
        return aug

    d_Wih = np.asarray(inp["d_Wih"], f32)
    d_bih = np.asarray(inp["d_bih"], f32)
    d_bhh = np.asarray(inp["d_bhh"], f32)
    dc_b = d_bih.copy()
    dc_b[0:2 * H] += d_bhh[0:2 * H]

    shared["em_w1t"] = np.ascontiguousarray(np.asarray(inp["em_W1"], f32).T)
    shared["em_b1row"] = np.asarray(inp["em_b1"], f32)[None, :]
    shared["em_w2t"] = np.ascontiguousarray(np.asarray(inp["em_W2"], f32).T)
    shared["em_b2row"] = np.asarray(inp["em_b2"], f32)[None, :]
    shared["eo_wt"] = np.ascontiguousarray(np.asarray(inp["eo_W"], f32).T)
    shared["eo_brow"] = np.asarray(inp["eo_b"], f32)[None, :]
    shared["dc_wt"] = np.ascontiguousarray(d_Wih[:, 0:2 * H].T)
    shared["dc_brow"] = dc_b[None, :]
    shared["dwy_t"] = np.ascontiguousarray(d_Wih[:, 2 * H:].T)
    shared["dwhh_t"] = np.ascontiguousarray(np.asarray(inp["d_Whh"], f32).T)
    shared["dbhhn_row"] = np.ascontiguousarray(d_bhh[None, 2 * H:])
    shared["dm_w1t"] = np.ascontiguousarray(np.asarray(inp["dm_W1"], f32).T)
    shared["dm_b1row"] = np.asarray(inp["dm_b1"], f32)[None, :]
    shared["dm_w2t"] = np.ascontiguousarray(np.asarray(inp["dm_W2"], f32).T)
    shared["dm_b2row"] = np.asarray(inp["dm_b2"], f32)[None, :]
    shared["do_wt"] = np.ascontiguousarray(np.asarray(inp["do_W"], f32).T)
    shared["do_brow"] = np.asarray(inp["do_b"], f32)[None, :]

    for j in range(NCORE):
        chain, half = j // 2, j % 2
        pre, rev, seq = chains[chain]
        T = seq.shape[1]
        s = seq[128 * half:128 * (half + 1)]          # [128, T, 64]
        xin = np.zeros((66, et, BE), f32)
        xin[64, :, :] = 1.0
        pad = et - T
        if pad:
            xin[65, 0:pad, :] = 1.0
        order = np.arange(T)[::-1] if rev else np.arange(T)
        xin[0:64, pad:, :] = s[:, order, :].transpose(2, 1, 0)
        m = dict(shared)
        m["xin"] = np.ascontiguousarray(xin.reshape(66, et * BE))
        m["wih_aug"] = wih_aug(pre)
        m["whh_t"] = np.ascontiguousarray(np.asarray(inp[pre + "_Whh"],
                                                     f32).T)
        m["bhhn_row"] = np.ascontiguousarray(
            np.asarray(inp[pre + "_bhh"], f32)[None, 2 * H:])
        xl = np.concatenate([x[16 * j:16 * j + 16, -1, :],
                             x[128 + 16 * j:128 + 16 * j + 16, -1, :]])
        m["xlast_t"] = np.ascontiguousarray(xl.T)
        in_maps.append(m)
    return in_maps


def unshard(results, hor=60):
    out = np.zeros((B, hor, NY), np.float32)
    for j in range(NCORE):
        o = results[j]["out"].reshape(BD, hor, NY)
        out[16 * j:16 * j + 16] = o[0:16]
        out[128 + 16 * j:128 + 16 * j + 16] = o[16:32]
    return out


_NC = None


def kernel(**inputs):
    global _NC
    from concourse.bass_utils import run_bass_kernel_spmd
    if _NC is None:
        _NC = build_nc()
    in_maps = shard_inputs(inputs)
    res = run_bass_kernel_spmd(_NC, in_maps, core_ids=list(range(NCORE)))
    return unshard(res.results)



# revision 15
# speedup vs baseline: 1.6237x; 1.6237x over previous
"""Trainium2 Bass kernel for the GRU autoencoder (v2).

Distribution (8 NeuronCores), as baseline:
  Encode : chain-parallel x batch-parallel. Core j handles GRU chain j//2
           (xf, xb, ef, eb) on batch half j%2 (BE=128 rows), uniform 100-step
           loop (x-chains padded with exact identity steps via BIG z-gate).
  Reshard: AllToAll (bf16) so core j decodes rows [16j:16j+16] u [128+16j:+16].
  Middle : per-shard MLP + decoder const precompute.
  Decode : 60 autoregressive steps on a 32-row shard.

v2 performance changes vs baseline:
  * All weights converted/blocked on HOST (bf16) and DMA'd directly to SBUF
    (no on-device staging copies) -> encode starts immediately.
  * Encoder: bf16 weights/state, PE group order (g3,g2,g1a,g1b) so the
    elementwise chain overlaps the tail of the MM block; bf16 elementwise
    (PSUM accumulation stays fp32); xin DMA batched 4 steps at a time.
  * Decoder/middle: BLOCKED layout [(group j, batch b32), cols]. All M=32
    matmuls are PSUM col-tiled (4 concurrent tiles -> ~3x PE throughput) and
    all elementwise/activation ops run on full 128 partitions (4x cheaper).
    Transposes are row-tiled (4 concurrent). Gate bank holds r|z|hn|xn in one
    PSUM bank.
"""

import sys

sys.path.insert(0, "/opt/trn_rl_repo")

import numpy as np

import concourse.bass as bass
import concourse.mybir as mybir
import concourse.tile as tile
from concourse import bacc
from concourse.masks import make_identity

dt = mybir.dt
AF = mybir.ActivationFunctionType
OP = mybir.AluOpType

B, TX, TY, NX, NY, H, HOR = 256, 50, 100, 64, 64, 512, 60
M1, M2 = 1024, 512
G = 3 * H
NCORE = 8
BE = 128   # encoder batch rows per core
BD = 32    # decoder batch rows per core
ET = 100   # uniform encoder step count
BIG = 30000.0

F32, BF16 = dt.float32, dt.bfloat16
BF = np.dtype(mybir.dt.np(BF16))


def build_nc(et=ET, hor=HOR):
    nc = bacc.Bacc("TRN2", target_bir_lowering=False, debug=False,
                   num_devices=NCORE)

    # ---- DRAM parameters (identical names on every core) -------------------
    d_xin = nc.dram_tensor("xin", [66, et * BE], BF16, kind="ExternalInput")
    d_wih = nc.dram_tensor("wih_aug", [66, G], BF16, kind="ExternalInput")
    d_whh = nc.dram_tensor("whh_t", [H, G], BF16, kind="ExternalInput")
    d_bhhn = nc.dram_tensor("bhhn_row", [1, H], BF16, kind="ExternalInput")

    d_dwhh = nc.dram_tensor("dwhh_blk", [H, G], BF16, kind="ExternalInput")
    d_dwyrz = nc.dram_tensor("dwy_rz", [96, 1024], BF16, kind="ExternalInput")
    d_dwyxn = nc.dram_tensor("dwy_xn", [96, 512], BF16, kind="ExternalInput")
    d_dbhhn = nc.dram_tensor("dbhhn_blk", [1, 512], BF16, kind="ExternalInput")
    d_dm1 = nc.dram_tensor("dm1_t", [H, M1], BF16, kind="ExternalInput")
    d_dm1b = nc.dram_tensor("dm1b_row", [1, M1], BF16, kind="ExternalInput")
    d_dm2 = nc.dram_tensor("dm2_t", [M1, M2], BF16, kind="ExternalInput")
    d_dm2b = nc.dram_tensor("dm2b_row", [1, M2], BF16, kind="ExternalInput")
    d_dow = nc.dram_tensor("dow_t", [M2, NY], BF16, kind="ExternalInput")
    d_dob = nc.dram_tensor("dob_row", [1, NY], BF16, kind="ExternalInput")
    d_xlastT = nc.dram_tensor("xlast_t", [NX, BD], BF16, kind="ExternalInput")

    d_em1x = nc.dram_tensor("em1x_t", [H, M1], BF16, kind="ExternalInput")
    d_em1y = nc.dram_tensor("em1y_t", [H, M1], BF16, kind="ExternalInput")
    d_em1b = nc.dram_tensor("em1b_row", [1, M1], BF16, kind="ExternalInput")
    d_em2 = nc.dram_tensor("em2_t", [M1, M2], BF16, kind="ExternalInput")
    d_em2b = nc.dram_tensor("em2b_row", [1, M2], BF16, kind="ExternalInput")
    d_eo = nc.dram_tensor("eo_t", [M2, H], BF16, kind="ExternalInput")
    d_eob = nc.dram_tensor("eob_row", [1, H], BF16, kind="ExternalInput")
    d_dcw = nc.dram_tensor("dcw_blk", [2 * H, G], BF16, kind="ExternalInput")
    d_dcb = nc.dram_tensor("dcb_blk", [1, G], BF16, kind="ExternalInput")

    d_out = nc.dram_tensor("out", [NY, hor * BD], F32, kind="ExternalOutput")

    cc_in = nc.dram_tensor("cc_in", [BE, H], BF16)
    cc_out = nc.dram_tensor("cc_out", [NCORE, 16, H], BF16)

    with tile.TileContext(nc) as tc:
        with tc.tile_pool(name="wts", bufs=1) as wp, \
             tc.tile_pool(name="xin", bufs=2) as xp, \
             tc.tile_pool(name="state", bufs=2) as st, \
             tc.tile_pool(name="chain", bufs=2) as ch, \
             tc.tile_pool(name="persist", bufs=1) as pe:

            # ---------- constants ----------
            idf = pe.tile([128, 128], F32, tag="idf")
            make_identity(nc, idf[:])
            idb = pe.tile([128, 128], BF16, tag="idb")
            nc.gpsimd.tensor_copy(idb[:], idf[:])
            ones_b = pe.tile([1, 128], BF16, tag="ones_b")
            nc.gpsimd.memset(ones_b[:], 1.0)

            def wload(dram_ap, rows, cols, tag):
                t = wp.tile([rows, cols], BF16, tag=tag)
                nc.sync.dma_start(t[:], dram_ap)
                return t

            # encoder weights first (needed immediately)
            wih = wload(d_wih[:], 66, G, "wih")
            whh = [wload(d_whh[128 * c:128 * (c + 1), :], 128, G, f"whh{c}")
                   for c in range(4)]
            bhhn = wload(d_bhhn[:], 1, H, "bhhn")

            # decoder/middle weights (DMA streams during encode)
            dwhh = [wload(d_dwhh[128 * c:128 * (c + 1), :], 128, G,
                          f"dwhh{c}") for c in range(4)]
            dwyrz = wload(d_dwyrz[:], 96, 1024, "dwyrz")
            dwyxn = wload(d_dwyxn[:], 96, 512, "dwyxn")
            dbhhn = wload(d_dbhhn[:], 1, 512, "dbhhn")
            dm1 = [wload(d_dm1[128 * c:128 * (c + 1), :], 128, M1,
                         f"dm1_{c}") for c in range(4)]
            dm1b = wload(d_dm1b[:], 1, M1, "dm1b")
            dm2 = [wload(d_dm2[128 * c:128 * (c + 1), :], 128, M2,
                         f"dm2_{c}") for c in range(8)]
            dm2b = wload(d_dm2b[:], 1, M2, "dm2b")
            dow = [wload(d_dow[128 * c:128 * (c + 1), :], 128, NY,
                         f"dow{c}") for c in range(4)]
            dob = wload(d_dob[:], 1, NY, "dob")
            em1x = [wload(d_em1x[128 * c:128 * (c + 1), :], 128, M1,
                          f"em1x{c}") for c in range(4)]
            em1y = [wload(d_em1y[128 * c:128 * (c + 1), :], 128, M1,
                          f"em1y{c}") for c in range(4)]
            em1b = wload(d_em1b[:], 1, M1, "em1b")
            em2 = [wload(d_em2[128 * c:128 * (c + 1), :], 128, M2,
                         f"em2_{c}") for c in range(8)]
            em2b = wload(d_em2b[:], 1, M2, "em2b")
            eo = [wload(d_eo[128 * c:128 * (c + 1), :], 128, H,
                        f"eo{c}") for c in range(4)]
            eob = wload(d_eob[:], 1, H, "eob")
            dcw = [wload(d_dcw[128 * c:128 * (c + 1), :], 128, G,
                         f"dcw{c}") for c in range(8)]
            dcb = wload(d_dcb[:], 1, G, "dcb")
            xlastT = wload(d_xlastT[:], NX, BD, "xlastT")

            # ---------- encoder state ----------
            h_b = pe.tile([BE, H], BF16, tag="h0")
            nc.gpsimd.memset(h_b[:], 0.0)
            hT = pe.tile([128, H], BF16, tag="hT0")
            nc.gpsimd.memset(hT[:], 0.0)

            # ================= ENCODE =================
            with tc.tile_pool(name="pg3", bufs=2, space="PSUM") as pg3, \
                 tc.tile_pool(name="pg2", bufs=1, space="PSUM") as pg2, \
                 tc.tile_pool(name="pga", bufs=1, space="PSUM") as pga, \
                 tc.tile_pool(name="pgb", bufs=1, space="PSUM") as pgb, \
                 tc.tile_pool(name="ptr", bufs=2, space="PSUM") as ptr, \
                 tc.tile_pool(name="pwm", bufs=1, space="PSUM") as pwm:

                warm = pwm.tile([128, 512], F32, tag="warm")

                def shadow(tile_ap, n=1):
                    # PE filler pinned behind a chain op: a dummy matmul whose
                    # stationary operand is that op's output, so it runs right
                    # after it and holds the HAM clock gate at 8/8 through the
                    # elementwise window. No consumers.
                    for _ in range(n):
                        nc.tensor.matmul(warm[:], tile_ap, whh[0][:, 0:512],
                                         start=True, stop=True)

                for t in range(et):
                    if t % 4 == 0:
                        xb = xp.tile([66, 4 * BE], BF16, tag="xb")
                        nc.sync.dma_start(
                            xb[:], d_xin[:, t * BE:(t + 4) * BE])
                    xs = xb[:, (t % 4) * BE:(t % 4 + 1) * BE]

                    # PE order: g1a (r first), g3, g2 (n-path), g1b (z)
                    g1a = pga.tile([BE, 512], F32, tag="g1a")
                    nc.tensor.matmul(g1a[:], xs, wih[:, 0:512],
                                     start=True, stop=False)
                    for c in range(4):
                        nc.tensor.matmul(g1a[:], hT[:, 128 * c:128 * (c + 1)],
                                         whh[c][:, 0:512],
                                         start=False, stop=(c == 3))
                    g2 = pg2.tile([BE, 512], F32, tag="g2")
                    for c in range(4):
                        nc.tensor.matmul(g2[:], hT[:, 128 * c:128 * (c + 1)],
                                         whh[c][:, 1024:1536],
                                         start=(c == 0), stop=False)
                    nc.tensor.matmul(g2[:], ones_b[0:1, 0:BE], bhhn[:],
                                     start=False, stop=True)
                    g3 = pg3.tile([BE, 512], F32, tag="g3")
                    nc.tensor.matmul(g3[:], xs, wih[:, 1024:1536],
                                     start=True, stop=True)
                    g1b = pgb.tile([BE, 512], F32, tag="g1b")
                    nc.tensor.matmul(g1b[:], xs, wih[:, 512:1024],
                                     start=True, stop=False)
                    for c in range(4):
                        nc.tensor.matmul(g1b[:], hT[:, 128 * c:128 * (c + 1)],
                                         whh[c][:, 512:1024],
                                         start=False, stop=(c == 3))

                    r_b = ch.tile([BE, 512], BF16, tag="r")
                    nc.scalar.activation(r_b[:], g1a[:], AF.Sigmoid)
                    shadow(r_b[:, 0:128])
                    rhn = ch.tile([BE, 512], BF16, tag="rhn")
                    nc.vector.tensor_mul(rhn[:], r_b[:], g2[:])
                    shadow(rhn[:, 0:128])
                    npre = ch.tile([BE, 512], BF16, tag="npre")
                    nc.vector.tensor_add(npre[:], rhn[:], g3[:])
                    shadow(npre[:, 0:128])
                    n_b = ch.tile([BE, 512], BF16, tag="n")
                    nc.scalar.activation(n_b[:], npre[:], AF.Tanh)
                    shadow(n_b[:, 0:128])
                    z_b = ch.tile([BE, 512], BF16, tag="z")
                    nc.scalar.activation(z_b[:], g1b[:], AF.Sigmoid)
                    shadow(z_b[:, 0:128])
                    # h' = n + z*(h - n)
                    d_t = ch.tile([BE, 512], BF16, tag="d")
                    nc.vector.tensor_sub(d_t[:], h_b[:], n_b[:])
                    shadow(d_t[:, 0:128])
                    zm = ch.tile([BE, 512], BF16, tag="zm")
                    nc.vector.tensor_mul(zm[:], z_b[:], d_t[:])
                    shadow(zm[:, 0:128])
                    h_new = st.tile([BE, H], BF16, tag="h")
                    for half in range(2):
                        sl = slice(256 * half, 256 * (half + 1))
                        nc.vector.tensor_add(h_new[:, sl], n_b[:, sl],
                                             zm[:, sl])
                    hT_new = st.tile([128, H], BF16, tag="hT")
                    for half in range(2):
                        p = ptr.tile([128, 256], BF16, tag="tr")
                        for c in range(2):
                            cc = 2 * half + c
                            nc.tensor.transpose(
                                p[:, 128 * c:128 * (c + 1)],
                                h_new[:, 128 * cc:128 * (cc + 1)], idb[:])
                        nc.vector.tensor_copy(
                            hT_new[:, 256 * half:256 * (half + 1)], p[:])
                    hT, h_b = hT_new, h_new

                nc.sync.dma_start(cc_in[:], h_b[:])

            # ================= RESHARD =================
            nc.gpsimd.collective_compute(
                "AllToAll", OP.bypass,
                replica_groups=[list(range(NCORE))],
                ins=[cc_in[:]], outs=[cc_out[:]])

            with tc.tile_pool(name="pgt", bufs=2, space="PSUM") as pgt, \
                 tc.tile_pool(name="pm1", bufs=1, space="PSUM") as pm1, \
                 tc.tile_pool(name="pm2", bufs=1, space="PSUM") as pm2, \
                 tc.tile_pool(name="ptr2", bufs=2, space="PSUM") as pt2, \
                 tc.tile_pool(name="pdo", bufs=1, space="PSUM") as pdo, \
                 tc.tile_pool(name="pwm2", bufs=1, space="PSUM") as pwm2:

                warm2 = pwm2.tile([128, 512], F32, tag="warm2")

                def shadow2(tile_ap, n=1, rows=128):
                    # PE filler pinned behind a chain op (see encode shadow).
                    kk = tile_ap.partition_size()
                    for _ in range(n):
                        nc.tensor.matmul(warm2[0:rows, :], tile_ap,
                                         dwhh[0][0:kk, 0:512],
                                         start=True, stop=True)

                # ---------- gather + h_x / h_y ----------
                pxa = ch.tile([BD, H], BF16, tag="pxa")
                pxb = ch.tile([BD, H], BF16, tag="pxb")
                pya = ch.tile([BD, H], BF16, tag="pya")
                pyb = ch.tile([BD, H], BF16, tag="pyb")
                nc.sync.dma_start(pxa[0:16, :], cc_out[0][:])
                nc.sync.dma_start(pxa[16:32, :], cc_out[1][:])
                nc.sync.dma_start(pxb[0:16, :], cc_out[2][:])
                nc.sync.dma_start(pxb[16:32, :], cc_out[3][:])
                nc.sync.dma_start(pya[0:16, :], cc_out[4][:])
                nc.sync.dma_start(pya[16:32, :], cc_out[5][:])
                nc.sync.dma_start(pyb[0:16, :], cc_out[6][:])
                nc.sync.dma_start(pyb[16:32, :], cc_out[7][:])
                hx = pe.tile([BD, H], BF16, tag="hx")
                hy = pe.tile([BD, H], BF16, tag="hy")
                nc.vector.tensor_add(hx[:], pxa[:], pxb[:])
                nc.vector.tensor_add(hy[:], pya[:], pyb[:])

                def trsp32(src_bf, tag):
                    """src [32, 512] (partition base 0) -> [128, 128] bf16,
                    h-dim chunk c -> cols 32c:32c+32."""
                    p = pt2.tile([128, 128], BF16, tag="trp")
                    for c in range(4):
                        nc.tensor.transpose(
                            p[:, 32 * c:32 * (c + 1)],
                            src_bf[0:32, 128 * c:128 * (c + 1)],
                            idb[0:32, 0:32])
                    o = pe.tile([128, 128], BF16, tag=tag)
                    nc.scalar.copy(o[:], p[:])
                    return o

                hxT = trsp32(hx, "hxT")
                hyT = trsp32(hy, "hyT")

                # ---------- m1 = relu(W1x hx + W1y hy + b1), blocked ------
                m1 = pm1.tile([128, 256], F32, tag="m1")
                for j in range(4):
                    for k in range(4):
                        nc.tensor.matmul(
                            m1[32 * j:32 * (j + 1), :], hxT[:, 32 * k:32 * (k + 1)],
                            em1x[k][:, 256 * j:256 * (j + 1)],
                            start=(k == 0), stop=False,
                            tile_position=(0, 32 * j))
                    for k in range(4):
                        nc.tensor.matmul(
                            m1[32 * j:32 * (j + 1), :], hyT[:, 32 * k:32 * (k + 1)],
                            em1y[k][:, 256 * j:256 * (j + 1)],
                            start=False, stop=False,
                            tile_position=(0, 32 * j))
                    nc.tensor.matmul(
                        m1[32 * j:32 * (j + 1), :], ones_b[0:1, 0:BD],
                        em1b[:, 256 * j:256 * (j + 1)], start=False, stop=True,
                        tile_position=(0, 32 * j))
                hm1 = ch.tile([128, 256], BF16, tag="hm1")
                nc.scalar.activation(hm1[:], m1[:], AF.Relu)

                def tr_m1(src_bf, tag):
                    """blocked [ (j,b32), 256 ] -> [128, 32*8] chunks of the
                    1024-dim; chunk k at dst col 32*perm[k]. Two full-tile
                    half transposes: left half -> even chunks, right -> odd."""
                    p = pt2.tile([128, 256], BF16, tag="trp")
                    nc.tensor.transpose(p[:, 0:128], src_bf[:, 0:128], idb[:])
                    nc.tensor.transpose(p[:, 128:256], src_bf[:, 128:256],
                                        idb[:])
                    perm = [0, 4, 1, 5, 2, 6, 3, 7]
                    o = pe.tile([128, 256], BF16, tag=tag)
                    nc.scalar.copy(o[:], p[:])
                    return o, perm

                hm1T, p1 = tr_m1(hm1, "hm1T")

                # ---------- m2 = relu(W2 m1 + b2), blocked ----------------
                m2 = pm2.tile([128, 128], F32, tag="m2")
                for j in range(4):
                    for k in range(8):
                        nc.tensor.matmul(
                            m2[32 * j:32 * (j + 1), :],
                            hm1T[:, 32 * p1[k]:32 * (p1[k] + 1)],
                            em2[k][:, 128 * j:128 * (j + 1)],
                            start=(k == 0), stop=False,
                            tile_position=(0, 32 * j))
                    nc.tensor.matmul(
                        m2[32 * j:32 * (j + 1), :], ones_b[0:1, 0:BD],
                        em2b[:, 128 * j:128 * (j + 1)], start=False, stop=True,
                        tile_position=(0, 32 * j))
                hm2 = ch.tile([128, 128], BF16, tag="hm2")
                nc.scalar.activation(hm2[:], m2[:], AF.Relu)

                def tr_sq(src_bf, tag):
                    """blocked [(j,b32), 128] -> [128, 128]; chunk j -> col 32j.
                    One full-tile transpose: out[:, 32j+b] = src[32j+b, :]."""
                    p = pt2.tile([128, 128], BF16, tag="trp")
                    nc.tensor.transpose(p[:], src_bf[:], idb[:])
                    o = pe.tile([128, 128], BF16, tag=tag)
                    nc.scalar.copy(o[:], p[:])
                    return o

                hm2T = tr_sq(hm2, "hm2T")

                # ---------- z = eo m2 + b, blocked ------------------------
                zp = pm2.tile([128, 128], F32, tag="m2")
                for j in range(4):
                    for k in range(4):
                        nc.tensor.matmul(
                            zp[32 * j:32 * (j + 1), :],
                            hm2T[:, 32 * k:32 * (k + 1)],
                            eo[k][:, 128 * j:128 * (j + 1)],
                            start=(k == 0), stop=False,
                            tile_position=(0, 32 * j))
                    nc.tensor.matmul(
                        zp[32 * j:32 * (j + 1), :], ones_b[0:1, 0:BD],
                        eob[:, 128 * j:128 * (j + 1)], start=False, stop=True,
                        tile_position=(0, 32 * j))
                z_b = ch.tile([128, 128], BF16, tag="z_b")
                nc.scalar.copy(z_b[:], zp[:])
                zT = tr_sq(z_b, "zT")

                # ---------- const = dcw [hx; z] + dcb, blocked ------------
                cst = pgt.tile([128, 512], F32, tag="g")
                for j in range(4):
                    for k in range(4):
                        nc.tensor.matmul(
                            cst[32 * j:32 * (j + 1), 0:384],
                            hxT[:, 32 * k:32 * (k + 1)],
                            dcw[k][:, 384 * j:384 * (j + 1)],
                            start=(k == 0), stop=False,
                            tile_position=(0, 32 * j))
                    for k in range(4):
                        nc.tensor.matmul(
                            cst[32 * j:32 * (j + 1), 0:384],
                            zT[:, 32 * k:32 * (k + 1)],
                            dcw[4 + k][:, 384 * j:384 * (j + 1)],
                            start=False, stop=False,
                            tile_position=(0, 32 * j))
                    nc.tensor.matmul(
                        cst[32 * j:32 * (j + 1), 0:384], ones_b[0:1, 0:BD],
                        dcb[:, 384 * j:384 * (j + 1)], start=False, stop=True,
                        tile_position=(0, 32 * j))
                cst_b = ch.tile([128, 384], BF16, tag="cst")
                nc.scalar.copy(cst_b[:], cst[:, 0:384])
                # rearrange const rows into dwyrz/dwyxn id-rows (64:96)
                for j in range(4):
                    nc.sync.dma_start(
                        dwyrz[64:96, 256 * j:256 * (j + 1)],
                        cst_b[32 * j:32 * (j + 1), 0:256])
                    nc.sync.dma_start(
                        dwyxn[64:96, 128 * j:128 * (j + 1)],
                        cst_b[32 * j:32 * (j + 1), 256:384])

                # ---------- decoder init ----------
                idhi96 = pe.tile([96, BD], BF16, tag="idhi96")
                nc.sync.dma_start(idhi96[64:96, :], idb[0:32, 0:32])
                ypT = st.tile([96, BD], BF16, tag="ypT")
                nc.vector.tensor_copy(ypT[0:NX, :], xlastT[:])
                nc.vector.tensor_copy(ypT[64:96, :], idhi96[64:96, :])
                hd_b = st.tile([128, 128], BF16, tag="hd")
                nc.gpsimd.memset(hd_b[:], 0.0)
                hdT = st.tile([128, 128], BF16, tag="hdT")
                nc.gpsimd.memset(hdT[:], 0.0)

                def emit_gates_pre(g, hdT_src):
                    """h-side + bhh_n matmuls (y-independent part of gates)."""
                    for k in range(4):
                        for j in range(4):
                            nc.tensor.matmul(
                                g[32 * j:32 * (j + 1), 0:384],
                                hdT_src[:, 32 * k:32 * (k + 1)],
                                dwhh[k][:, 384 * j:384 * (j + 1)],
                                start=(k == 0), stop=False,
                                tile_position=(0, 32 * j))
                    for j in range(4):
                        nc.tensor.matmul(
                            g[32 * j:32 * (j + 1), 256:384], ones_b[0:1, 0:BD],
                            dbhhn[:, 128 * j:128 * (j + 1)],
                            start=False, stop=True,
                            tile_position=(0, 32 * j))

                P1 = [0, 4, 1, 5, 2, 6, 3, 7]
                g_cur = pgt.tile([128, 512], F32, tag="g")
                emit_gates_pre(g_cur, hdT)

                # ================= DECODE =================
                for t in range(hor):
                    g = g_cur
                    # y_p-side (closes rz and xn accumulation groups)
                    for j in range(4):
                        nc.tensor.matmul(
                            g[32 * j:32 * (j + 1), 0:256], ypT[:],
                            dwyrz[:, 256 * j:256 * (j + 1)],
                            start=False, stop=True,
                            tile_position=(0, 32 * j))
                    for j in range(4):
                        nc.tensor.matmul(
                            g[32 * j:32 * (j + 1), 384:512], ypT[:],
                            dwyxn[:, 128 * j:128 * (j + 1)],
                            start=True, stop=True,
                            tile_position=(0, 32 * j))
                    rz = ch.tile([128, 256], BF16, tag="rz")
                    nc.scalar.activation(rz[:], g[:, 0:256], AF.Sigmoid)
                    shadow2(rz[:, 0:128])
                    rhn = ch.tile([128, 128], BF16, tag="drhn")
                    nc.vector.tensor_mul(rhn[:], rz[:, 0:128], g[:, 256:384])
                    shadow2(rhn[:])
                    npre = ch.tile([128, 128], BF16, tag="dnpre")
                    nc.vector.tensor_add(npre[:], rhn[:], g[:, 384:512])
                    shadow2(npre[:])
                    n_b = ch.tile([128, 128], BF16, tag="dn")
                    nc.scalar.activation(n_b[:], npre[:], AF.Tanh)
                    shadow2(n_b[:])
                    d_t = ch.tile([128, 128], BF16, tag="dd")
                    nc.vector.tensor_sub(d_t[:], hd_b[:], n_b[:])
                    shadow2(d_t[:])
                    zm = ch.tile([128, 128], BF16, tag="dzm")
                    nc.vector.tensor_mul(zm[:], rz[:, 128:256], d_t[:])
                    shadow2(zm[:])
                    hd_new = st.tile([128, 128], BF16, tag="hd")
                    nc.vector.tensor_add(hd_new[:], n_b[:], zm[:])

                    ptd = pt2.tile([128, 128], BF16, tag="trp")
                    nc.tensor.transpose(ptd[:], hd_new[:], idb[:])
                    hdT_new = st.tile([128, 128], BF16, tag="hdT")
                    nc.scalar.copy(hdT_new[:], ptd[:])
                    hdT, hd_b = hdT_new, hd_new

                    # ---- m1 ----
                    m1 = pm1.tile([128, 256], F32, tag="m1")
                    for k in range(4):
                        for j in range(4):
                            nc.tensor.matmul(
                                m1[32 * j:32 * (j + 1), :],
                                hdT[:, 32 * k:32 * (k + 1)],
                                dm1[k][:, 256 * j:256 * (j + 1)],
                                start=(k == 0), stop=False,
                                tile_position=(0, 32 * j))
                    for j in range(4):
                        nc.tensor.matmul(
                            m1[32 * j:32 * (j + 1), :], ones_b[0:1, 0:BD],
                            dm1b[:, 256 * j:256 * (j + 1)],
                            start=False, stop=True,
                            tile_position=(0, 32 * j))
                    # pre-emit next step's y-independent gate matmuls: they
                    # stream on PE while this step's m1 relu/transpose runs.
                    if t + 1 < hor:
                        g_cur = pgt.tile([128, 512], F32, tag="g")
                        emit_gates_pre(g_cur, hdT)

                    hm1 = ch.tile([128, 256], BF16, tag="hm1")
                    p1t = pt2.tile([128, 256], BF16, tag="trp")
                    hm1T = ch.tile([128, 256], BF16, tag="hm1T")
                    for hf in range(2):
                        sl = slice(128 * hf, 128 * (hf + 1))
                        nc.scalar.activation(hm1[:, sl], m1[:, sl], AF.Relu)
                        nc.tensor.transpose(p1t[:, sl], hm1[:, sl], idb[:])
                        nc.vector.tensor_copy(hm1T[:, sl], p1t[:, sl])
                        shadow2(hm1[:, sl])

                    # ---- m2 (even chunks first: left half of hm1T) ----
                    m2 = pm2.tile([128, 128], F32, tag="m2")
                    for ki, k in enumerate([0, 2, 4, 6, 1, 3, 5, 7]):
                        for j in range(4):
                            nc.tensor.matmul(
                                m2[32 * j:32 * (j + 1), :],
                                hm1T[:, 32 * P1[k]:32 * (P1[k] + 1)],
                                dm2[k][:, 128 * j:128 * (j + 1)],
                                start=(ki == 0), stop=False,
                                tile_position=(0, 32 * j))
                    for j in range(4):
                        nc.tensor.matmul(
                            m2[32 * j:32 * (j + 1), :], ones_b[0:1, 0:BD],
                            dm2b[:, 128 * j:128 * (j + 1)],
                            start=False, stop=True,
                            tile_position=(0, 32 * j))
                    hm2 = ch.tile([128, 128], BF16, tag="hm2")
                    nc.scalar.activation(hm2[:], m2[:], AF.Relu)
                    shadow2(hm2[:])
                    p2t = pt2.tile([128, 128], BF16, tag="trp")
                    nc.tensor.transpose(p2t[:], hm2[:], idb[:])
                    hm2T = ch.tile([128, 128], BF16, tag="hm2T")
                    nc.vector.tensor_copy(hm2T[:], p2t[:])
                    shadow2(hm2T[:])

                    # ---- do (transposed): yT = Wo @ hm2.T directly ----
                    ytp = pdo.tile([NY, BD], F32, tag="do")
                    for k in range(4):
                        nc.tensor.matmul(ytp[:], dow[k][:, 0:NY],
                                         hm2T[:, 32 * k:32 * (k + 1)],
                                         start=(k == 0), stop=False)
                    nc.tensor.matmul(ytp[:], dob[:], ones_b[0:1, 0:BD],
                                     start=False, stop=True)
                    y_sb = ch.tile([NY, BD], F32, tag="y_sb")
                    nc.vector.tensor_copy(y_sb[:], ytp[:])
                    nc.sync.dma_start(d_out[:, BD * t:BD * (t + 1)], y_sb[:])
                    if t + 1 < hor:
                        ypT_new = st.tile([96, BD], BF16, tag="ypT")
                        nc.scalar.copy(ypT_new[0:NX, :], ytp[:])
                        shadow2(ypT_new[0:NX, :], rows=32)
                        nc.vector.tensor_copy(ypT_new[64:96, :],
                                              idhi96[64:96, :])
                        ypT = ypT_new

    nc.compile()
    return nc


# ---------------------------------------------------------------------------
# Host-side sharding
# ---------------------------------------------------------------------------

# gate-block permutation: blocked col j*384 + s*128 + c  <- gate row 512s+128j+c
_IDXG = np.array([512 * s + 128 * j + c
                  for j in range(4) for s in range(3) for c in range(128)])
_IDXRZ = np.array([512 * s + 128 * j + c
                   for j in range(4) for s in range(2) for c in range(128)])


def shard_inputs(inp, et=ET, hor=HOR):
    f32 = np.float32
    x, y = np.asarray(inp["x"], f32), np.asarray(inp["y"], f32)
    chains = [("xf", False, x), ("xb", True, x),
              ("ef", False, y), ("eb", True, y)]
    in_maps = []
    shared = {}

    def bf(a):
        return np.ascontiguousarray(np.asarray(a, f32).astype(BF))

    def wih_aug(pre):
        wih = np.asarray(inp[pre + "_Wih"], f32)
        bih = np.asarray(inp[pre + "_bih"], f32)
        bhh = np.asarray(inp[pre + "_bhh"], f32)
        aug = np.zeros((66, G), f32)
        aug[0:64, :] = wih.T
        bias = bih.copy()
        bias[0:2 * H] += bhh[0:2 * H]
        aug[64, :] = bias
        aug[65, H:2 * H] = BIG
        return bf(aug)

    d_Wih = np.asarray(inp["d_Wih"], f32)
    d_bih = np.asarray(inp["d_bih"], f32)
    d_bhh = np.asarray(inp["d_bhh"], f32)

    # decoder GRU weights, gate-blocked
    shared["dwhh_blk"] = bf(np.asarray(inp["d_Whh"], f32).T[:, _IDXG])
    wyT = d_Wih[:, 2 * H:].T                      # [64, 1536]
    dwyrz = np.zeros((96, 1024), f32)
    dwyrz[0:64, :] = wyT[:, _IDXRZ]
    shared["dwy_rz"] = bf(dwyrz)
    idxn = np.array([1024 + 128 * j + c for j in range(4) for c in range(128)])
    dwyxn = np.zeros((96, 512), f32)
    dwyxn[0:64, :] = wyT[:, idxn]
    shared["dwy_xn"] = bf(dwyxn)
    shared["dbhhn_blk"] = bf(d_bhh[None, 2 * H:])
    dcb = d_bih.copy()
    dcb[0:2 * H] += d_bhh[0:2 * H]
    shared["dcw_blk"] = bf(d_Wih[:, 0:2 * H].T[:, _IDXG])
    shared["dcb_blk"] = bf(dcb[None, _IDXG])

    shared["dm1_t"] = bf(np.asarray(inp["dm_W1"], f32).T)
    shared["dm1b_row"] = bf(np.asarray(inp["dm_b1"], f32)[None, :])
    shared["dm2_t"] = bf(np.asarray(inp["dm_W2"], f32).T)
    shared["dm2b_row"] = bf(np.asarray(inp["dm_b2"], f32)[None, :])
    shared["dow_t"] = bf(np.asarray(inp["do_W"], f32).T)
    shared["dob_row"] = bf(np.asarray(inp["do_b"], f32)[None, :])

    em_W1 = np.asarray(inp["em_W1"], f32)
    shared["em1x_t"] = bf(em_W1[:, 0:H].T)
    shared["em1y_t"] = bf(em_W1[:, H:].T)
    shared["em1b_row"] = bf(np.asarray(inp["em_b1"], f32)[None, :])
    shared["em2_t"] = bf(np.asarray(inp["em_W2"], f32).T)
    shared["em2b_row"] = bf(np.asarray(inp["em_b2"], f32)[None, :])
    shared["eo_t"] = bf(np.asarray(inp["eo_W"], f32).T)
    shared["eob_row"] = bf(np.asarray(inp["eo_b"], f32)[None, :])

    for j in range(NCORE):
        chain, half = j // 2, j % 2
        pre, rev, seq = chains[chain]
        T = seq.shape[1]
        s = seq[128 * half:128 * (half + 1)]          # [128, T, 64]
        xin = np.zeros((66, et, BE), f32)
        xin[64, :, :] = 1.0
        pad = et - T
        if pad:
            xin[65, 0:pad, :] = 1.0
        order = np.arange(T)[::-1] if rev else np.arange(T)
        xin[0:64, pad:, :] = s[:, order, :].transpose(2, 1, 0)
        m = dict(shared)
        m["xin"] = bf(xin.reshape(66, et * BE))
        m["wih_aug"] = wih_aug(pre)
        m["whh_t"] = bf(np.asarray(inp[pre + "_Whh"], f32).T)
        m["bhhn_row"] = bf(np.asarray(inp[pre + "_bhh"], f32)[None, 2 * H:])
        xl = np.concatenate([x[16 * j:16 * j + 16, -1, :],
                             x[128 + 16 * j:128 + 16 * j + 16, -1, :]])
        m["xlast_t"] = bf(xl.T)
        in_maps.append(m)
    return in_maps


def unshard(results, hor=HOR):
    out = np.zeros((B, hor, NY), np.float32)
    for j in range(NCORE):
        o = results[j]["out"].reshape(NY, hor, BD).transpose(2, 1, 0)
        out[16 * j:16 * j + 16] = o[0:16]
        out[128 + 16 * j:128 + 16 * j + 16] = o[16:32]
    return out


_NC = None


def kernel(**inputs):
    global _NC
    from concourse.bass_utils import run_bass_kernel_spmd
    if _NC is None:
        _NC = build_nc()
    in_maps = shard_inputs(inputs)
    res = run_bass_kernel_spmd(_NC, in_maps, core_ids=list(range(NCORE)))
    return unshard(res.results)
